# revision 1
# baseline (speedup 1.0000x reference)
"""Trainium2 Bass kernel for nn_DepthwiseXCorr (SiamRPN-style depthwise-xcorr head).

Pipeline per sample (B=128 sharded 16/core across 8 cores, pure data parallel):
  k = relu(bn1(conv3x3(kernel_in, w_ck)))      [256, 5, 5]
  s = relu(bn2(conv3x3(search_in, w_cs)))      [256, 29, 29]
  feat = depthwise_xcorr(s, k)                 [256, 25, 25]
  h = relu(bn3(conv1x1(feat, w_h1)))           [256, 25, 25]
  out = conv1x1(h, w_h2) + b_h2                [10, 25, 25]

Implementation notes:
  - BN scale is folded into conv weights host-side; BN shift + ReLU applied by
    the ACT engine on the PSUM->SBUF eviction (activation = relu(x*1 + bias)).
  - Convs are implicit GEMM on TensorE: input channels (128/chunk) on
    partitions, conv taps accumulate in PSUM, dtype float32r (full PE rate at
    moving free dim >= 256).
  - Depthwise xcorr is a per-channel 5x5 correlation: 25 fused multiply-add
    (scalar_tensor_tensor) ops with per-partition scalars, split DVE/GpSimd.
"""

import numpy as np

EPS = 1e-5
N_CORES = 8
B = 128
B_PER = B // N_CORES  # 16
CIN = 256
H = 256
COUT = 10

_NC_CACHE = {}


def _build_nc(b_per=B_PER, pe_mod=4, gp_pat=(0, 3, 5, 8, 10)):
    """Build the Bass program for one core processing `b_per` samples.

    xcorr routing: every (sample, channel-chunk) unit with index % pe_mod == 0
    runs on the PE via diagonal-weight matmuls; remaining units are split into
    row-halves distributed DVE/GpSimd (halves whose running index mod 12 is in
    gp_pat go to GpSimd).
    """
    import concourse.bacc as bacc
    import concourse.mybir as mybir
    import concourse.tile as tile

    dt = mybir.dt
    f32 = dt.float32
    f32r = dt.float32r
    AF = mybir.ActivationFunctionType
    ALU = mybir.AluOpType

    nc = bacc.Bacc("TRN2", target_bir_lowering=False, debug=False)

    # ---- DRAM tensors (shapes match SBUF tiles exactly; host pre-transposes) ----
    search_d = nc.dram_tensor("search", [b_per, 256, 31, 31], f32r, kind="ExternalInput")
    kin_d = nc.dram_tensor("kin", [2, 128, 9, b_per, 25], f32r, kind="ExternalInput")
    wk_d = nc.dram_tensor("wk", [2, 128, 18, 128], f32r, kind="ExternalInput")
    ws_d = nc.dram_tensor("ws", [2, 128, 18, 128], f32r, kind="ExternalInput")
    w1_d = nc.dram_tensor("w1", [2, 128, 2, 128], f32r, kind="ExternalInput")
    w2_d = nc.dram_tensor("w2", [2, 128, 10], f32r, kind="ExternalInput")
    eye_d = nc.dram_tensor("eye", [128, 128], f32, kind="ExternalInput")
    b1_d = nc.dram_tensor("b1s", [128, 2], f32, kind="ExternalInput")
    b2_d = nc.dram_tensor("b2s", [128, 2], f32, kind="ExternalInput")
    b3_d = nc.dram_tensor("b3s", [128, 2], f32, kind="ExternalInput")
    bh_d = nc.dram_tensor("bhs", [10, 1], f32, kind="ExternalInput")
    y_d = nc.dram_tensor("y", [b_per, 10, 25, 25], f32, kind="ExternalOutput")

    TAPS3 = [(dy, dx) for dy in range(3) for dx in range(3)]
    TAPS5 = [(dy, dx) for dy in range(5) for dx in range(5)]
    # conv_search output row tiling: 29 rows -> two PSUM tiles (N = 435 / 406)
    CS_ROWS = [(0, 15), (15, 14)]
    # h1/h2 output position tiling: 25 rows -> two PSUM tiles (N = 325 / 300)
    H_ROWS = [(0, 13), (13, 12)]

    with tile.TileContext(nc) as tc:
        with (
            tc.tile_pool(name="wpool", bufs=1) as wpool,
            tc.tile_pool(name="kpool", bufs=1) as kpool,
            tc.tile_pool(name="spool", bufs=6) as spool,
            tc.tile_pool(name="fpool", bufs=16) as fpool,
            tc.tile_pool(name="hpool", bufs=6) as hpool,
            tc.tile_pool(name="sfpool", bufs=8) as sfpool,
            tc.tile_pool(name="opool", bufs=3) as opool,
            tc.tile_pool(name="dpool", bufs=3) as dpool,
            tc.tile_pool(name="ps_cs", bufs=3, space="PSUM") as ps_cs,
            tc.tile_pool(name="ps_h", bufs=5, space="PSUM") as ps_h,
        ):
            # ---- conv_kernel inputs first (unblocks phase K quickly) ----
            kin_sb = []
            for c in range(2):
                kt = kpool.tile([128, 9, b_per, 25], f32r, tag=f"kin{c}")
                (nc.scalar if c == 0 else nc.sync).dma_start(kt[:], kin_d[c])
                kin_sb.append(kt)

            # ---- load weights / biases (persistent) ----
            wk_sb = []
            ws_sb = []
            w1_sb = []
            w2_sb = []
            for c in range(2):
                wkt = wpool.tile([128, 18, 128], f32r, tag=f"wk{c}")
                nc.sync.dma_start(wkt[:], wk_d[c])
                wk_sb.append(wkt)
                wst = wpool.tile([128, 18, 128], f32r, tag=f"ws{c}")
                nc.scalar.dma_start(wst[:], ws_d[c])
                ws_sb.append(wst)
                w1t = wpool.tile([128, 2, 128], f32r, tag=f"w1{c}")
                nc.sync.dma_start(w1t[:], w1_d[c])
                w1_sb.append(w1t)
                w2t = wpool.tile([128, 10], f32r, tag=f"w2{c}")
                nc.scalar.dma_start(w2t[:], w2_d[c])
                w2_sb.append(w2t)
            b1_sb = wpool.tile([128, 2], f32, tag="b1")
            nc.sync.dma_start(b1_sb[:], b1_d[:])
            b2_sb = wpool.tile([128, 2], f32, tag="b2")
            nc.sync.dma_start(b2_sb[:], b2_d[:])
            b3_sb = wpool.tile([128, 2], f32, tag="b3")
            nc.sync.dma_start(b3_sb[:], b3_d[:])
            bh_sb = wpool.tile([10, 1], f32, tag="bh")
            nc.sync.dma_start(bh_sb[:], bh_d[:])
            eye_sb = wpool.tile([128, 128], f32, tag="eye")
            nc.scalar.dma_start(eye_sb[:], eye_d[:])

            # ---- phase K: conv_kernel for all samples batched (N = b_per*25) ----
            kf_sb = []
            for cc in range(2):  # output-channel chunk
                psk = ps_cs.tile([128, b_per, 25], f32, tag="ps")
                n_acc = len(TAPS3) * 2
                i = 0
                for (dy, dx) in TAPS3:
                    for ci in range(2):
                        t2c = (dy * 3 + dx) * 2 + cc
                        nc.tensor.matmul(
                            psk[:],
                            wk_sb[ci][:, t2c, :],
                            kin_sb[ci][:, dy * 3 + dx, :, :],
                            start=(i == 0),
                            stop=(i == n_acc - 1),
                        )
                        i += 1
                kf = kpool.tile([128, b_per, 25], f32, tag=f"kf{cc}")
                nc.scalar.activation(kf[:], psk[:], AF.Relu, bias=b1_sb[:, cc : cc + 1])
                kf_sb.append(kf)

            # ---- per-sample pipeline ----
            PE_UNITS = tuple(u for u in range(2 * b_per)
                             if u >= 4 and u % 2 == 0)
            diag_tiles = {}

            def build_diag(u):
                bb, cc = u // 2, u % 2
                dg = dpool.tile([128, 25, 128], f32r, tag="diag")
                # dg[c, t, j] = eye[c, j] * k[c, t]: writes the full tile
                # (zeros off-diagonal), so slot rotation is safe
                mask = eye_sb[:].unsqueeze(1).broadcast_to([128, 25, 128])
                data = kf_sb[cc][:, bb].unsqueeze(2).broadcast_to([128, 25, 128])
                nc.gpsimd.tensor_tensor(dg[:], mask, data, ALU.mult)
                diag_tiles[u] = dg

            unit_idx = 0
            pe_cnt = 0
            half_cnt = 0
            for b in range(b_per):
                # build diagonal weights one sample ahead of their PE unit
                for u in PE_UNITS:
                    if u // 2 == b + 1:
                        build_diag(u)
                # load search input (2 channel chunks)
                sin = []
                for ci in range(2):
                    st = spool.tile([128, 31, 32], f32r, tag="sin")
                    q = (nc.sync, nc.scalar)[(b * 2 + ci) % 2]
                    q.dma_start(st[:, :, 0:31], search_d[b, ci * 128 : (ci + 1) * 128])
                    sin.append(st)

                # conv_search + bn2 + relu -> s_feat [2][128, 29, 29]
                sf = []
                for cc in range(2):
                    sft = sfpool.tile([128, 29, 30], f32r, tag="sf")
                    for (r0, nr) in CS_ROWS:
                        pscs = ps_cs.tile([128, 15, 30], f32, tag="ps")
                        n_acc = len(TAPS3) * 2
                        i = 0
                        for (dy, dx) in TAPS3:
                            t2c = (dy * 3 + dx) * 2 + cc
                            for ci in range(2):
                                nc.tensor.matmul(
                                    pscs[:, :nr, :],
                                    ws_sb[ci][:, t2c, :],
                                    sin[ci][:, dy + r0 : dy + r0 + nr, dx : dx + 30],
                                    start=(i == 0),
                                    stop=(i == n_acc - 1),
                                )
                                i += 1
                        nc.scalar.activation(
                            sft[:, r0 : r0 + nr, 0:29],
                            pscs[:, :nr, 0:29],
                            AF.Relu,
                            bias=b2_sb[:, cc : cc + 1],
                        )
                    sf.append(sft)

                # depthwise xcorr -> feat [cc][half][128, <=13, 25]
                feat = [[None, None], [None, None]]
                for cc in range(2):
                    on_pe = unit_idx in PE_UNITS
                    if on_pe:
                        pe_cnt += 1
                        if unit_idx not in diag_tiles:
                            build_diag(unit_idx)
                        dg = diag_tiles.pop(unit_idx)
                        for hi, (r0, nr) in enumerate(H_ROWS):
                            psx = ps_h.tile([128, 13, 26], f32, tag="ph")
                            for ti, (dy, dx) in enumerate(TAPS5):
                                nc.tensor.matmul(
                                    psx[:, :nr, :],
                                    dg[:, ti, :],
                                    sf[cc][:, dy + r0 : dy + r0 + nr, dx : dx + 26],
                                    start=(ti == 0),
                                    stop=(ti == 24),
                                )
                            ft = fpool.tile([128, 13, 26], f32r, tag="feat")
                            nc.scalar.activation(
                                ft[:, :nr, 0:25], psx[:, :nr, 0:25], AF.Copy
                            )
                            feat[cc][hi] = ft
                    else:
                        for hi, (r0, nr) in enumerate(H_ROWS):
                            eng = nc.vector
                            half_cnt += 1
                            ft = fpool.tile([128, 13, 26], f32r, tag="feat")
                            for ti, (dy, dx) in enumerate(TAPS5):
                                kap = kf_sb[cc][:, b, dy * 5 + dx : dy * 5 + dx + 1]
                                win = sf[cc][:, dy + r0 : dy + r0 + nr, dx : dx + 25]
                                if ti == 0:
                                    eng.tensor_scalar(
                                        ft[:, :nr, 0:25], win, kap, None, ALU.mult
                                    )
                                else:
                                    eng.scalar_tensor_tensor(
                                        ft[:, :nr, 0:25], win, kap, ft[:, :nr, 0:25],
                                        ALU.mult, ALU.add,
                                    )
                            feat[cc][hi] = ft
                    unit_idx += 1

                # h1: 1x1 conv + bn3 + relu -> h1o [2][128, 25, 25]
                h1o = []
                for cc2 in range(2):
                    ht = hpool.tile([128, 25, 26], f32r, tag="h1o")
                    for hi, (r0, nr) in enumerate(H_ROWS):
                        psh = ps_h.tile([128, 13, 26], f32, tag="ph")
                        for ci in range(2):
                            nc.tensor.matmul(
                                psh[:, :nr, :],
                                w1_sb[ci][:, cc2, :],
                                feat[ci][hi][:, :nr, :],
                                start=(ci == 0),
                                stop=(ci == 1),
                            )
                        nc.scalar.activation(
                            ht[:, r0 : r0 + nr, 0:25],
                            psh[:, :nr, 0:25],
                            AF.Relu,
                            bias=b3_sb[:, cc2 : cc2 + 1],
                        )
                    h1o.append(ht)

                # h2: 1x1 conv (+bias) -> out [10, 25, 25]
                osb = opool.tile([10, 25, 25], f32, tag="osb")
                for (r0, nr) in H_ROWS:
                    psh2 = ps_h.tile([10, 13, 26], f32, tag="ph")
                    for ci in range(2):
                        nc.tensor.matmul(
                            psh2[:, :nr, :],
                            w2_sb[ci][:, :],
                            h1o[ci][:, r0 : r0 + nr, :],
                            start=(ci == 0),
                            stop=(ci == 1),
                        )
                    nc.scalar.activation(
                        osb[:, r0 : r0 + nr, :],
                        psh2[:, :nr, 0:25],
                        AF.Identity,
                        bias=bh_sb[:, :],
                    )
                nc.sync.dma_start(y_d[b], osb[:])

    nc.compile()
    return nc


def _get_nc(b_per=B_PER):
    key = b_per
    if key not in _NC_CACHE:
        _NC_CACHE[key] = _build_nc(b_per)
    return _NC_CACHE[key]


def _host_prep(inputs):
    """Fold BN into weights, transpose to lhsT layouts, slice per core."""
    f = np.float32
    kernel = np.ascontiguousarray(inputs["kernel"], dtype=f)
    search = np.ascontiguousarray(inputs["search"], dtype=f)

    def bn_fold(g, b_, m, v):
        scale = g / np.sqrt(v + EPS)
        shift = b_ - m * scale
        return scale.astype(f), shift.astype(f)

    s1, sh1 = bn_fold(inputs["g1"], inputs["b1"], inputs["m1"], inputs["v1"])
    s2, sh2 = bn_fold(inputs["g2"], inputs["b2"], inputs["m2"], inputs["v2"])
    s3, sh3 = bn_fold(inputs["g3"], inputs["b3"], inputs["m3"], inputs["v3"])

    def conv3_lhsT(w, scale):
        # w [co=256, ci=256, 3, 3] * scale[co] -> [cic, ci128, tap*2+coc, co128]
        wf = (w * scale[:, None, None, None]).astype(f)
        wf = wf.reshape(2, 128, 2, 128, 3, 3)  # [coc, co, cic, ci, dy, dx]
        wf = wf.transpose(2, 3, 4, 5, 0, 1)  # [cic, ci, dy, dx, coc, co]
        return np.ascontiguousarray(wf.reshape(2, 128, 18, 128))

    wk = conv3_lhsT(inputs["w_ck"], s1)
    ws = conv3_lhsT(inputs["w_cs"], s2)

    w1 = (inputs["w_h1"][:, :, 0, 0] * s3[:, None]).astype(f)  # [co 256, ci 256]
    w1 = w1.reshape(2, 128, 2, 128).transpose(2, 3, 0, 1)  # [cic, ci, coc, co]
    w1 = np.ascontiguousarray(w1)
    w2 = inputs["w_h2"][:, :, 0, 0].astype(f)  # [10, 256]
    w2 = np.ascontiguousarray(w2.reshape(10, 2, 128).transpose(1, 2, 0))  # [cic, ci, 10]

    weights = dict(
        wk=wk,
        ws=ws,
        w1=w1,
        w2=w2,
        eye=np.eye(128, dtype=f),
        b1s=np.ascontiguousarray(sh1.reshape(2, 128).T),
        b2s=np.ascontiguousarray(sh2.reshape(2, 128).T),
        b3s=np.ascontiguousarray(sh3.reshape(2, 128).T),
        bhs=np.ascontiguousarray(inputs["b_h2"].astype(f).reshape(10, 1)),
    )

    in_maps = []
    for c in range(N_CORES):
        sl = slice(c * B_PER, (c + 1) * B_PER)
        win = np.lib.stride_tricks.sliding_window_view(kernel[sl], (5, 5), axis=(2, 3))
        # win[b, c, dy, dx, y, x] = kernel[b, c, y+dy, x+dx]
        kin = win.reshape(B_PER, 2, 128, 9, 25).transpose(1, 2, 3, 0, 4)
        m = dict(weights)
        m["search"] = search[sl]
        m["kin"] = np.ascontiguousarray(kin)
        in_maps.append(m)
    return in_maps


def run(trace=False, **inputs):
    from concourse import bass_utils

    in_maps = _host_prep(inputs)
    nc = _get_nc()
    try:
        res = bass_utils.run_bass_kernel_spmd(
            nc, in_maps, core_ids=list(range(N_CORES)), trace=trace
        )
    except ModuleNotFoundError:
        # NTFF profiling hook unavailable in this container
        res = bass_utils.run_bass_kernel_spmd(
            nc, in_maps, core_ids=list(range(N_CORES)), trace=False
        )
    y = np.concatenate([res.results[c]["y"] for c in range(N_CORES)], axis=0)
    return y.reshape(B, 10, 25, 25), res


def kernel(**inputs):
    y, _ = run(trace=False, **inputs)
    return y



# revision 41
# speedup vs baseline: 1.0593x; 1.0593x over previous
"""Trainium2 Bass kernel for nn_DepthwiseXCorr (SiamRPN-style depthwise-xcorr head).

Pipeline per sample (B=128 sharded 16/core across 8 cores, pure data parallel):
  k = relu(bn1(conv3x3(kernel_in, w_ck)))      [256, 5, 5]
  s = relu(bn2(conv3x3(search_in, w_cs)))      [256, 29, 29]
  feat = depthwise_xcorr(s, k)                 [256, 25, 25]
  h = relu(bn3(conv1x1(feat, w_h1)))           [256, 25, 25]
  out = conv1x1(h, w_h2) + b_h2                [10, 25, 25]

Implementation notes:
  - BN scale folded into conv weights host-side; BN shift + ReLU applied by
    the ACT engine on the PSUM->SBUF eviction (activation = relu(x*1 + bias)).
  - Convs are implicit GEMM on TensorE (fp32r runs 1 cycle/row at free dim
    >= 256, same rate as bf16).
  - Depthwise xcorr (32 (sample,chunk) units/core) is load-balanced across
    engines with an explicit map: DVE and Pool run 25-tap
    scalar_tensor_tensor chains; 7 units (incl. both units of the last
    sample) run on the PE as diagonal-weight matmuls, their diag tiles built
    by the otherwise-idle ACT engine two samples ahead.
  - The head (h1/h2) is software-pipelined 2 samples behind conv/xcorr so
    slow Pool units never stall the PE (PE p-state drops on any stall).
  - DMA ring discipline: search loads on the SP ring only, y stores on the
    ACT ring (issued right after the osb eviction so they never wait),
    weights split across the ACT/DVE rings at startup.
"""

import numpy as np

EPS = 1e-5
N_CORES = 8
B = 128
B_PER = B // N_CORES  # 16
CIN = 256
H = 256
COUT = 10

_NC_CACHE = {}

# unit u = sample*2 + chunk.  GPSIMD supports no scalar_tensor_tensor on real
# HW, so the depthwise xcorr is split PE/DVE only: 14 units run on the PE as
# diagonal-weight matmuls (diag tiles built by Pool broadcast multiplies), 18
# units run as 25-tap scalar_tensor_tensor chains on DVE.
PE_UNITS = (3, 5, 7, 9, 11, 13, 15, 17, 19, 21, 23, 25, 27, 30, 31)


def _build_nc(b_per=B_PER):
    """Build the Bass program for one core processing `b_per` samples."""
    import concourse.bacc as bacc
    import concourse.mybir as mybir
    import concourse.tile as tile

    dt = mybir.dt
    f32 = dt.float32
    f32r = dt.float32r
    AF = mybir.ActivationFunctionType
    ALU = mybir.AluOpType

    nc = bacc.Bacc("TRN2", target_bir_lowering=False, debug=False)

    # ---- DRAM tensors (shapes match SBUF tiles exactly; host pre-transposes) ----
    search_d = nc.dram_tensor("search", [b_per, 2, 128, 31, 32], f32r, kind="ExternalInput")
    kin_d = nc.dram_tensor("kin", [2, 128, 9, b_per, 25], dt.bfloat16, kind="ExternalInput")
    wk_d = nc.dram_tensor("wk", [2, 128, 18, 128], dt.bfloat16, kind="ExternalInput")
    ws_d = nc.dram_tensor("ws", [2, 128, 18, 128], f32r, kind="ExternalInput")
    w1_d = nc.dram_tensor("w1", [2, 128, 2, 128], f32r, kind="ExternalInput")
    w2_d = nc.dram_tensor("w2", [2, 128, 10], f32r, kind="ExternalInput")
    eye_d = nc.dram_tensor("eye", [128, 128], f32, kind="ExternalInput")
    b1_d = nc.dram_tensor("b1s", [128, 2], f32, kind="ExternalInput")
    b2_d = nc.dram_tensor("b2s", [128, 2], f32, kind="ExternalInput")
    b3_d = nc.dram_tensor("b3s", [128, 2], f32, kind="ExternalInput")
    bh_d = nc.dram_tensor("bhs", [10, 1], f32, kind="ExternalInput")
    y_d = nc.dram_tensor("y", [b_per, 10, 25, 25], f32, kind="ExternalOutput")

    TAPS3 = [(dy, dx) for dy in range(3) for dx in range(3)]
    TAPS5 = [(dy, dx) for dy in range(5) for dx in range(5)]
    # conv_search output row tiling: 29 rows -> two PSUM tiles (N = 435 / 406)
    CS_ROWS = [(0, 15), (15, 14)]
    # h1/h2/PE-xcorr output row tiling: 25 rows -> two PSUM tiles (N = 325 / 300)
    H_ROWS = [(0, 13), (13, 12)]

    with tile.TileContext(nc) as tc:
        with (
            tc.tile_pool(name="wpool", bufs=1) as wpool,
            tc.tile_pool(name="kpool", bufs=1) as kpool,
            tc.tile_pool(name="spool", bufs=6) as spool,
            tc.tile_pool(name="fpool", bufs=10) as fpool,
            tc.tile_pool(name="hpool", bufs=4) as hpool,
            tc.tile_pool(name="sfpool", bufs=8) as sfpool,
            tc.tile_pool(name="opool", bufs=3) as opool,
            tc.tile_pool(name="dpool", bufs=2) as dpool,
            tc.tile_pool(name="ps_cs", bufs=2, space="PSUM") as ps_cs,
            tc.tile_pool(name="ps_h", bufs=4, space="PSUM") as ps_h,
            tc.tile_pool(name="ps_h2", bufs=2, space="PSUM") as ps_h2,
        ):
            sin_tiles = {}

            def emit_load(b):
                sin = []
                for ci in range(2):
                    st = spool.tile([128, 31, 32], f32r, tag="sin")
                    nc.sync.dma_start(st[:], search_d[b, ci])
                    sin.append(st)
                sin_tiles[b] = sin

            # ---- startup DMA schedule, ordered by first use:
            #   ACT ring:   kraw0+wk0 (phase K ci=0) + b1, then per-sample y
            #   SP ring:    sin[0], kraw1+wk1 (phase K ci=1), sin[1], head
            #               weights, then the sin prefetch stream
            #   Pool ring:  ws (conv_search) + b2 + eye ----
            kin_sb = []
            wk_sb = []
            ws_sb = []
            w1_sb = []
            w2_sb = []
            # transfers serialize on the DMA engines, so order strictly by
            # first use: conv_search[0] inputs, then phase-K inputs, then the
            # second sample and head weights
            b2_sb = wpool.tile([128, 2], f32, tag="b2")
            nc.scalar.dma_start(b2_sb[:], b2_d[:])
            for c in range(2):
                wst = wpool.tile([128, 18, 128], f32r, tag=f"ws{c}")
                nc.gpsimd.dma_start(wst[:], ws_d[c])
                ws_sb.append(wst)
            emit_load(0)
            for c in range(2):
                kt = kpool.tile([128, 9, b_per, 25], dt.bfloat16, tag=f"kin{c}")
                nc.scalar.dma_start(kt[:], kin_d[c])
                kin_sb.append(kt)
                wkt = wpool.tile([128, 18, 128], dt.bfloat16, tag=f"wk{c}")
                nc.gpsimd.dma_start(wkt[:], wk_d[c])
                wk_sb.append(wkt)
            b1_sb = wpool.tile([128, 2], f32, tag="b1")
            nc.scalar.dma_start(b1_sb[:], b1_d[:])
            emit_load(1)
            eye_sb = wpool.tile([128, 128], f32, tag="eye")
            nc.gpsimd.dma_start(eye_sb[:], eye_d[:])
            for c in range(2):
                w1t = wpool.tile([128, 2, 128], f32r, tag=f"w1{c}")
                nc.sync.dma_start(w1t[:], w1_d[c])
                w1_sb.append(w1t)
                w2t = wpool.tile([128, 10], f32r, tag=f"w2{c}")
                nc.sync.dma_start(w2t[:], w2_d[c])
                w2_sb.append(w2t)
            b3_sb = wpool.tile([128, 2], f32, tag="b3")
            nc.sync.dma_start(b3_sb[:], b3_d[:])
            bh_sb = wpool.tile([10, 1], f32, tag="bh")
            nc.sync.dma_start(bh_sb[:], bh_d[:])

            # ---- phase K: conv_kernel for all samples batched (N = b_per*25).
            # Emitted lazily AFTER conv_search[0] so the PE starts on conv
            # while the (larger) kin transfers are still in flight. ----
            kf_sb = []

            def emit_phase_k():
                for cc in range(2):  # output-channel chunk
                    psk = ps_cs.tile([128, b_per, 25], f32, tag="ps")
                    n_acc = len(TAPS3) * 2
                    i = 0
                    for ci in range(2):
                        for (dy, dx) in TAPS3:
                            t2c = (dy * 3 + dx) * 2 + cc
                            nc.tensor.matmul(
                                psk[:],
                                wk_sb[ci][:, t2c, :],
                                kin_sb[ci][:, dy * 3 + dx, :, :],
                                start=(i == 0),
                                stop=(i == n_acc - 1),
                            )
                            i += 1
                    kf = kpool.tile([128, b_per, 25], f32, tag=f"kf{cc}")
                    nc.scalar.activation(kf[:], psk[:], AF.Relu, bias=b1_sb[:, cc : cc + 1])
                    kf_sb.append(kf)

            diag_tiles = {}

            def build_diag(u):
                """diag(kf[:, b, t]) for all 25 taps: dg[c, t, j] = eye[c,j]*kf[c,b,t].

                One Pool broadcast tensor_tensor (~6.5us); Pool is otherwise
                idle so this never contends with evictions or DVE chains."""
                bb, cc = u // 2, u % 2
                dg = dpool.tile([128, 25, 128], f32r, tag="diag")
                mask = eye_sb[:].unsqueeze(1).broadcast_to([128, 25, 128])
                data = kf_sb[cc][:, bb].unsqueeze(2).broadcast_to([128, 25, 128])
                nc.gpsimd.tensor_tensor(dg[:], mask, data, ALU.mult)
                diag_tiles[u] = dg

            sf_tiles = {}
            feat_tiles = {}

            def emit_convs(b, mid_hook=None):
                sin = sin_tiles.pop(b)
                sf = []
                for cc in range(2):
                    if cc == 1 and mid_hook is not None:
                        mid_hook()
                    sft = sfpool.tile([128, 29, 30], f32r, tag="sf")
                    for (r0, nr) in CS_ROWS:
                        pscs = ps_cs.tile([128, 15, 30], f32, tag="ps")
                        n_acc = len(TAPS3) * 2
                        i = 0
                        for ci in range(2):
                            for (dy, dx) in TAPS3:
                                t2c = (dy * 3 + dx) * 2 + cc
                                nc.tensor.matmul(
                                    pscs[:, :nr, :],
                                    ws_sb[ci][:, t2c, :],
                                    sin[ci][:, dy + r0 : dy + r0 + nr, dx : dx + 30],
                                    start=(i == 0),
                                    stop=(i == n_acc - 1),
                                )
                                i += 1
                        nc.scalar.activation(
                            sft[:, r0 : r0 + nr, 0:29],
                            pscs[:, :nr, 0:29],
                            AF.Relu,
                            bias=b2_sb[:, cc : cc + 1],
                        )
                    sf.append(sft)
                sf_tiles[b] = sf

            def emit_xcorr(b):
                sf = sf_tiles.pop(b)
                feat = [None, None]
                for cc in range(2):
                    u = b * 2 + cc
                    ft = fpool.tile([128, 25, 26], f32r, tag="feat")
                    if u in PE_UNITS:
                        dg = diag_tiles.pop(u)
                        for (r0, nr) in H_ROWS:
                            psx = ps_h.tile([128, 13, 26], f32, tag="ph")
                            for ti, (dy, dx) in enumerate(TAPS5):
                                nc.tensor.matmul(
                                    psx[:, :nr, :],
                                    dg[:, ti, :],
                                    sf[cc][:, dy + r0 : dy + r0 + nr, dx : dx + 26],
                                    start=(ti == 0),
                                    stop=(ti == 24),
                                )
                            nc.scalar.activation(
                                ft[:, r0 : r0 + nr, 0:25], psx[:, :nr, 0:25], AF.Copy
                            )
                    else:
                        dv = ft[:, :, 0:25]
                        for ti, (dy, dx) in enumerate(TAPS5):
                            kap = kf_sb[cc][:, b, dy * 5 + dx : dy * 5 + dx + 1]
                            win = sf[cc][:, dy : dy + 25, dx : dx + 25]
                            if ti == 0:
                                nc.vector.tensor_scalar(dv, win, kap, None, ALU.mult)
                            else:
                                nc.vector.scalar_tensor_tensor(
                                    dv, win, kap, dv, ALU.mult, ALU.add
                                )
                    feat[cc] = ft

                feat_tiles[b] = feat

            def emit_head(b):
                feat = feat_tiles.pop(b)
                # h1: 1x1 conv + bn3 + relu -> h1o [2][128, 25, 25]
                h1o = []
                for cc2 in range(2):
                    ht = hpool.tile([128, 25, 26], f32r, tag="h1o")
                    for (r0, nr) in H_ROWS:
                        psh = ps_h.tile([128, 13, 26], f32, tag="ph")
                        for ci in range(2):
                            nc.tensor.matmul(
                                psh[:, :nr, :],
                                w1_sb[ci][:, cc2, :],
                                feat[ci][:, r0 : r0 + nr, :],
                                start=(ci == 0),
                                stop=(ci == 1),
                            )
                        nc.scalar.activation(
                            ht[:, r0 : r0 + nr, 0:25],
                            psh[:, :nr, 0:25],
                            AF.Relu,
                            bias=b3_sb[:, cc2 : cc2 + 1],
                        )
                    h1o.append(ht)

                # h2: 1x1 conv (+bias) -> out [10, 25, 25]
                osb = opool.tile([10, 25, 25], f32, tag="osb")
                for (r0, nr) in H_ROWS:
                    psh2 = ps_h2.tile([10, 13, 26], f32, tag="ph2")
                    for ci in range(2):
                        nc.tensor.matmul(
                            psh2[:, :nr, :],
                            w2_sb[ci][:, :],
                            h1o[ci][:, r0 : r0 + nr, :],
                            start=(ci == 0),
                            stop=(ci == 1),
                        )
                    nc.scalar.activation(
                        osb[:, r0 : r0 + nr, :],
                        psh2[:, :nr, 0:25],
                        AF.Identity,
                        bias=bh_sb[:, :],
                    )
                # y store on the ACT ring: osb was just evicted by ACT, so the
                # DMA dispatch never waits on a semaphore
                nc.scalar.dma_start(y_d[b], osb[:])

            # ---- software-pipelined sample loop (head lags by 2 samples).
            # The head is emitted BEFORE conv_search so its PSUM evictions sit
            # ahead of conv evictions in the in-order ACT queue (no
            # head-of-line blocking -> no PE stall on ps_h banks). ----
            # diag build schedule: 2 iterations ahead of use; when a sample
            # has two PE units, stagger the second to the next iteration so
            # only 2 diag tiles are ever live (dpool bufs=2)
            diag_sched = {}
            seen_samples = set()
            for u in PE_UNITS:
                it = u // 2 - 2
                if u // 2 in seen_samples:
                    it += 1
                seen_samples.add(u // 2)
                diag_sched.setdefault(max(it, 0), []).append(u)

            for b in range(b_per):
                if b + 2 < b_per:
                    emit_load(b + 2)
                emit_convs(b, mid_hook=emit_phase_k if b == 0 else None)
                for u in diag_sched.get(b, ()):
                    build_diag(u)
                emit_xcorr(b)
                if b >= 3:
                    emit_head(b - 3)
            emit_head(b_per - 3)
            emit_head(b_per - 2)
            emit_head(b_per - 1)

    nc.compile()
    return nc


def _get_nc(b_per=B_PER):
    key = b_per
    if key not in _NC_CACHE:
        _NC_CACHE[key] = _build_nc(b_per)
    return _NC_CACHE[key]


def _host_prep(inputs):
    """Fold BN into weights, transpose to lhsT layouts, slice per core."""
    import ml_dtypes

    bf16 = ml_dtypes.bfloat16
    f = np.float32
    kernel = np.ascontiguousarray(inputs["kernel"], dtype=f)
    search = np.ascontiguousarray(inputs["search"], dtype=f)

    def bn_fold(g, b_, m, v):
        scale = g / np.sqrt(v + EPS)
        shift = b_ - m * scale
        return scale.astype(f), shift.astype(f)

    s1, sh1 = bn_fold(inputs["g1"], inputs["b1"], inputs["m1"], inputs["v1"])
    s2, sh2 = bn_fold(inputs["g2"], inputs["b2"], inputs["m2"], inputs["v2"])
    s3, sh3 = bn_fold(inputs["g3"], inputs["b3"], inputs["m3"], inputs["v3"])

    def conv3_lhsT(w, scale):
        # w [co=256, ci=256, 3, 3] * scale[co] -> [cic, ci128, tap*2+coc, co128]
        wf = (w * scale[:, None, None, None]).astype(f)
        wf = wf.reshape(2, 128, 2, 128, 3, 3)  # [coc, co, cic, ci, dy, dx]
        wf = wf.transpose(2, 3, 4, 5, 0, 1)  # [cic, ci, dy, dx, coc, co]
        return np.ascontiguousarray(wf.reshape(2, 128, 18, 128))

    wk = conv3_lhsT(inputs["w_ck"], s1).astype(bf16)
    ws = conv3_lhsT(inputs["w_cs"], s2)

    w1 = (inputs["w_h1"][:, :, 0, 0] * s3[:, None]).astype(f)  # [co 256, ci 256]
    w1 = w1.reshape(2, 128, 2, 128).transpose(2, 3, 0, 1)  # [cic, ci, coc, co]
    w1 = np.ascontiguousarray(w1)
    w2 = inputs["w_h2"][:, :, 0, 0].astype(f)  # [10, 256]
    w2 = np.ascontiguousarray(w2.reshape(10, 2, 128).transpose(1, 2, 0))  # [cic, ci, 10]

    weights = dict(
        wk=wk,
        ws=ws,
        w1=w1,
        w2=w2,
        eye=np.eye(128, dtype=f),
        b1s=np.ascontiguousarray(sh1.reshape(2, 128).T),
        b2s=np.ascontiguousarray(sh2.reshape(2, 128).T),
        b3s=np.ascontiguousarray(sh3.reshape(2, 128).T),
        bhs=np.ascontiguousarray(inputs["b_h2"].astype(f).reshape(10, 1)),
    )

    in_maps = []
    for c in range(N_CORES):
        sl = slice(c * B_PER, (c + 1) * B_PER)
        win = np.lib.stride_tricks.sliding_window_view(kernel[sl], (5, 5), axis=(2, 3))
        # win[b, c, dy, dx, y, x] = kernel[b, c, y+dy, x+dx]
        kin = win.reshape(B_PER, 2, 128, 9, 25).transpose(1, 2, 3, 0, 4)
        sp = np.zeros((B_PER, 2, 128, 31, 32), dtype=f)
        sp[..., :31] = search[sl].reshape(B_PER, 2, 128, 31, 31)
        m = dict(weights)
        m["search"] = sp
        m["kin"] = np.ascontiguousarray(kin).astype(bf16)
        in_maps.append(m)
    return in_maps


def run(trace=False, **inputs):
    from concourse import bass_utils

    in_maps = _host_prep(inputs)
    nc = _get_nc()
    try:
        res = bass_utils.run_bass_kernel_spmd(
            nc, in_maps, core_ids=list(range(N_CORES)), trace=trace
        )
    except ModuleNotFoundError:
        # NTFF profiling hook unavailable in this container
        res = bass_utils.run_bass_kernel_spmd(
            nc, in_maps, core_ids=list(range(N_CORES)), trace=False
        )
    y = np.concatenate([res.results[c]["y"] for c in range(N_CORES)], axis=0)
    return y.reshape(B, 10, 25, 25), res


def kernel(**inputs):
    y, _ = run(trace=False, **inputs)
    return y


# revision 42
# speedup vs baseline: 1.1124x; 1.0502x over previous
"""Trainium2 Bass kernel for nn_DepthwiseXCorr (SiamRPN-style depthwise-xcorr head).

Pipeline per sample (B=128 sharded 16/core across 8 cores, pure data parallel):
  k = relu(bn1(conv3x3(kernel_in, w_ck)))      [256, 5, 5]
  s = relu(bn2(conv3x3(search_in, w_cs)))      [256, 29, 29]
  feat = depthwise_xcorr(s, k)                 [256, 25, 25]
  h = relu(bn3(conv1x1(feat, w_h1)))           [256, 25, 25]
  out = conv1x1(h, w_h2) + b_h2                [10, 25, 25]

Implementation notes:
  - BN scale folded into conv weights host-side; BN shift + ReLU applied by
    the ACT engine on the PSUM->SBUF eviction (activation = relu(x*1 + bias)).
  - Convs are implicit GEMM on TensorE (fp32r runs 1 cycle/row at free dim
    >= 256, same rate as bf16).
  - Depthwise xcorr (32 (sample,chunk) units/core) is load-balanced across
    engines with an explicit map: DVE and Pool run 25-tap
    scalar_tensor_tensor chains; 7 units (incl. both units of the last
    sample) run on the PE as diagonal-weight matmuls, their diag tiles built
    by the otherwise-idle ACT engine two samples ahead.
  - The head (h1/h2) is software-pipelined 2 samples behind conv/xcorr so
    slow Pool units never stall the PE (PE p-state drops on any stall).
  - DMA ring discipline: search loads on the SP ring only, y stores on the
    ACT ring (issued right after the osb eviction so they never wait),
    weights split across the ACT/DVE rings at startup.
"""

import numpy as np

EPS = 1e-5
N_CORES = 8
B = 128
B_PER = B // N_CORES  # 16
CIN = 256
H = 256
COUT = 10

_NC_CACHE = {}

# unit u = sample*2 + chunk.  GPSIMD supports no scalar_tensor_tensor on real
# HW, so the depthwise xcorr is split PE/DVE only: 14 units run on the PE as
# diagonal-weight matmuls (diag tiles built by Pool broadcast multiplies), 18
# units run as 25-tap scalar_tensor_tensor chains on DVE.
PE_UNITS = (3, 5, 7, 9, 11, 13, 15, 17, 19, 21, 23, 25, 27, 30, 31)


def _build_nc(b_per=B_PER):
    """Build the Bass program for one core processing `b_per` samples."""
    import concourse.bacc as bacc
    import concourse.mybir as mybir
    import concourse.tile as tile

    dt = mybir.dt
    f32 = dt.float32
    f32r = dt.float32r
    AF = mybir.ActivationFunctionType
    ALU = mybir.AluOpType

    nc = bacc.Bacc("TRN2", target_bir_lowering=False, debug=False)

    # ---- DRAM tensors (shapes match SBUF tiles exactly; host pre-transposes) ----
    search_d = nc.dram_tensor("search", [b_per, 2, 128, 31, 32], dt.bfloat16, kind="ExternalInput")
    kin_d = nc.dram_tensor("kin", [2, 128, 9, b_per, 25], dt.bfloat16, kind="ExternalInput")
    wk_d = nc.dram_tensor("wk", [2, 128, 18, 128], dt.bfloat16, kind="ExternalInput")
    ws_d = nc.dram_tensor("ws", [2, 128, 18, 128], dt.bfloat16, kind="ExternalInput")
    w1_d = nc.dram_tensor("w1", [2, 128, 2, 128], f32r, kind="ExternalInput")
    w2_d = nc.dram_tensor("w2", [2, 128, 10], f32r, kind="ExternalInput")
    eye_d = nc.dram_tensor("eye", [128, 128], f32, kind="ExternalInput")
    b1_d = nc.dram_tensor("b1s", [128, 2], f32, kind="ExternalInput")
    b2_d = nc.dram_tensor("b2s", [128, 2], f32, kind="ExternalInput")
    b3_d = nc.dram_tensor("b3s", [128, 2], f32, kind="ExternalInput")
    bh_d = nc.dram_tensor("bhs", [10, 1], f32, kind="ExternalInput")
    y_d = nc.dram_tensor("y", [b_per, 10, 25, 25], f32, kind="ExternalOutput")

    TAPS3 = [(dy, dx) for dy in range(3) for dx in range(3)]
    TAPS5 = [(dy, dx) for dy in range(5) for dx in range(5)]
    # conv_search output row tiling: 29 rows -> two PSUM tiles (N = 435 / 406)
    CS_ROWS = [(0, 15), (15, 14)]
    # h1/h2/PE-xcorr output row tiling: 25 rows -> two PSUM tiles (N = 325 / 300)
    H_ROWS = [(0, 13), (13, 12)]

    with tile.TileContext(nc) as tc:
        with (
            tc.tile_pool(name="wpool", bufs=1) as wpool,
            tc.tile_pool(name="kpool", bufs=1) as kpool,
            tc.tile_pool(name="spool", bufs=6) as spool,
            tc.tile_pool(name="fpool", bufs=10) as fpool,
            tc.tile_pool(name="hpool", bufs=4) as hpool,
            tc.tile_pool(name="sfpool", bufs=8) as sfpool,
            tc.tile_pool(name="opool", bufs=3) as opool,
            tc.tile_pool(name="dpool", bufs=2) as dpool,
            tc.tile_pool(name="ps_cs", bufs=2, space="PSUM") as ps_cs,
            tc.tile_pool(name="ps_h", bufs=4, space="PSUM") as ps_h,
            tc.tile_pool(name="ps_h2", bufs=2, space="PSUM") as ps_h2,
        ):
            sin_tiles = {}

            def emit_load(b):
                sin = []
                for ci in range(2):
                    st = spool.tile([128, 31, 32], dt.bfloat16, tag="sin")
                    nc.sync.dma_start(st[:], search_d[b, ci])
                    sin.append(st)
                sin_tiles[b] = sin

            # ---- startup DMA schedule, ordered by first use:
            #   ACT ring:   kraw0+wk0 (phase K ci=0) + b1, then per-sample y
            #   SP ring:    sin[0], kraw1+wk1 (phase K ci=1), sin[1], head
            #               weights, then the sin prefetch stream
            #   Pool ring:  ws (conv_search) + b2 + eye ----
            kin_sb = []
            wk_sb = []
            ws_sb = []
            w1_sb = []
            w2_sb = []
            # transfers serialize on the DMA engines, so order strictly by
            # first use: conv_search[0] inputs, then phase-K inputs, then the
            # second sample and head weights
            b2_sb = wpool.tile([128, 2], f32, tag="b2")
            nc.scalar.dma_start(b2_sb[:], b2_d[:])
            for c in range(2):
                wst = wpool.tile([128, 18, 128], dt.bfloat16, tag=f"ws{c}")
                nc.gpsimd.dma_start(wst[:], ws_d[c])
                ws_sb.append(wst)
            emit_load(0)
            for c in range(2):
                kt = kpool.tile([128, 9, b_per, 25], dt.bfloat16, tag=f"kin{c}")
                nc.scalar.dma_start(kt[:], kin_d[c])
                kin_sb.append(kt)
                wkt = wpool.tile([128, 18, 128], dt.bfloat16, tag=f"wk{c}")
                nc.gpsimd.dma_start(wkt[:], wk_d[c])
                wk_sb.append(wkt)
            b1_sb = wpool.tile([128, 2], f32, tag="b1")
            nc.scalar.dma_start(b1_sb[:], b1_d[:])
            emit_load(1)
            eye_sb = wpool.tile([128, 128], f32, tag="eye")
            nc.gpsimd.dma_start(eye_sb[:], eye_d[:])
            for c in range(2):
                w1t = wpool.tile([128, 2, 128], f32r, tag=f"w1{c}")
                nc.sync.dma_start(w1t[:], w1_d[c])
                w1_sb.append(w1t)
                w2t = wpool.tile([128, 10], f32r, tag=f"w2{c}")
                nc.sync.dma_start(w2t[:], w2_d[c])
                w2_sb.append(w2t)
            b3_sb = wpool.tile([128, 2], f32, tag="b3")
            nc.sync.dma_start(b3_sb[:], b3_d[:])
            bh_sb = wpool.tile([10, 1], f32, tag="bh")
            nc.sync.dma_start(bh_sb[:], bh_d[:])

            # ---- phase K: conv_kernel for all samples batched (N = b_per*25).
            # Emitted lazily AFTER conv_search[0] so the PE starts on conv
            # while the (larger) kin transfers are still in flight. ----
            kf_sb = []

            def emit_phase_k():
                for cc in range(2):  # output-channel chunk
                    psk = ps_cs.tile([128, b_per, 25], f32, tag="ps")
                    n_acc = len(TAPS3) * 2
                    i = 0
                    for ci in range(2):
                        for (dy, dx) in TAPS3:
                            t2c = (dy * 3 + dx) * 2 + cc
                            nc.tensor.matmul(
                                psk[:],
                                wk_sb[ci][:, t2c, :],
                                kin_sb[ci][:, dy * 3 + dx, :, :],
                                start=(i == 0),
                                stop=(i == n_acc - 1),
                            )
                            i += 1
                    kf = kpool.tile([128, b_per, 25], f32, tag=f"kf{cc}")
                    nc.scalar.activation(kf[:], psk[:], AF.Relu, bias=b1_sb[:, cc : cc + 1])
                    kf_sb.append(kf)

            diag_tiles = {}

            def build_diag(u):
                """diag(kf[:, b, t]) for all 25 taps: dg[c, t, j] = eye[c,j]*kf[c,b,t].

                One Pool broadcast tensor_tensor (~6.5us); Pool is otherwise
                idle so this never contends with evictions or DVE chains."""
                bb, cc = u // 2, u % 2
                dg = dpool.tile([128, 25, 128], f32r, tag="diag")
                mask = eye_sb[:].unsqueeze(1).broadcast_to([128, 25, 128])
                data = kf_sb[cc][:, bb].unsqueeze(2).broadcast_to([128, 25, 128])
                nc.gpsimd.tensor_tensor(dg[:], mask, data, ALU.mult)
                diag_tiles[u] = dg

            sf_tiles = {}
            feat_tiles = {}

            def emit_convs(b, mid_hook=None):
                sin = sin_tiles.pop(b)
                sf = []
                for cc in range(2):
                    if cc == 1 and mid_hook is not None:
                        mid_hook()
                    sft = sfpool.tile([128, 29, 30], f32r, tag="sf")
                    for (r0, nr) in CS_ROWS:
                        pscs = ps_cs.tile([128, 15, 29], f32, tag="ps")
                        n_acc = len(TAPS3) * 2
                        i = 0
                        for ci in range(2):
                            for (dy, dx) in TAPS3:
                                t2c = (dy * 3 + dx) * 2 + cc
                                nc.tensor.matmul(
                                    pscs[:, :nr, :],
                                    ws_sb[ci][:, t2c, :],
                                    sin[ci][:, dy + r0 : dy + r0 + nr, dx : dx + 29],
                                    start=(i == 0),
                                    stop=(i == n_acc - 1),
                                )
                                i += 1
                        nc.scalar.activation(
                            sft[:, r0 : r0 + nr, 0:29],
                            pscs[:, :nr, :],
                            AF.Relu,
                            bias=b2_sb[:, cc : cc + 1],
                        )
                    sf.append(sft)
                sf_tiles[b] = sf

            def emit_xcorr(b):
                sf = sf_tiles.pop(b)
                feat = [None, None]
                for cc in range(2):
                    u = b * 2 + cc
                    ft = fpool.tile([128, 25, 26], f32r, tag="feat")
                    if u in PE_UNITS:
                        dg = diag_tiles.pop(u)
                        for (r0, nr) in H_ROWS:
                            psx = ps_h.tile([128, 13, 26], f32, tag="ph")
                            for ti, (dy, dx) in enumerate(TAPS5):
                                nc.tensor.matmul(
                                    psx[:, :nr, :],
                                    dg[:, ti, :],
                                    sf[cc][:, dy + r0 : dy + r0 + nr, dx : dx + 26],
                                    start=(ti == 0),
                                    stop=(ti == 24),
                                )
                            nc.scalar.activation(
                                ft[:, r0 : r0 + nr, 0:25], psx[:, :nr, 0:25], AF.Copy
                            )
                    else:
                        dv = ft[:, :, 0:25]
                        for ti, (dy, dx) in enumerate(TAPS5):
                            kap = kf_sb[cc][:, b, dy * 5 + dx : dy * 5 + dx + 1]
                            win = sf[cc][:, dy : dy + 25, dx : dx + 25]
                            if ti == 0:
                                nc.vector.tensor_scalar(dv, win, kap, None, ALU.mult)
                            else:
                                nc.vector.scalar_tensor_tensor(
                                    dv, win, kap, dv, ALU.mult, ALU.add
                                )
                    feat[cc] = ft

                feat_tiles[b] = feat

            def emit_head(b):
                feat = feat_tiles.pop(b)
                # h1: 1x1 conv + bn3 + relu -> h1o [2][128, 25, 25]
                h1o = []
                for cc2 in range(2):
                    ht = hpool.tile([128, 25, 26], f32r, tag="h1o")
                    for (r0, nr) in H_ROWS:
                        psh = ps_h.tile([128, 13, 26], f32, tag="ph")
                        for ci in range(2):
                            nc.tensor.matmul(
                                psh[:, :nr, :],
                                w1_sb[ci][:, cc2, :],
                                feat[ci][:, r0 : r0 + nr, :],
                                start=(ci == 0),
                                stop=(ci == 1),
                            )
                        nc.scalar.activation(
                            ht[:, r0 : r0 + nr, 0:25],
                            psh[:, :nr, 0:25],
                            AF.Relu,
                            bias=b3_sb[:, cc2 : cc2 + 1],
                        )
                    h1o.append(ht)

                # h2: 1x1 conv (+bias) -> out [10, 25, 25]
                osb = opool.tile([10, 25, 25], f32, tag="osb")
                for (r0, nr) in H_ROWS:
                    psh2 = ps_h2.tile([10, 13, 26], f32, tag="ph2")
                    for ci in range(2):
                        nc.tensor.matmul(
                            psh2[:, :nr, :],
                            w2_sb[ci][:, :],
                            h1o[ci][:, r0 : r0 + nr, :],
                            start=(ci == 0),
                            stop=(ci == 1),
                        )
                    nc.scalar.activation(
                        osb[:, r0 : r0 + nr, :],
                        psh2[:, :nr, 0:25],
                        AF.Identity,
                        bias=bh_sb[:, :],
                    )
                # y store on the ACT ring: osb was just evicted by ACT, so the
                # DMA dispatch never waits on a semaphore
                nc.scalar.dma_start(y_d[b], osb[:])

            # ---- software-pipelined sample loop (head lags by 2 samples).
            # The head is emitted BEFORE conv_search so its PSUM evictions sit
            # ahead of conv evictions in the in-order ACT queue (no
            # head-of-line blocking -> no PE stall on ps_h banks). ----
            # diag build schedule: 2 iterations ahead of use; when a sample
            # has two PE units, stagger the second to the next iteration so
            # only 2 diag tiles are ever live (dpool bufs=2)
            diag_sched = {}
            seen_samples = set()
            for u in PE_UNITS:
                it = u // 2 - 2
                if u // 2 in seen_samples:
                    it += 1
                seen_samples.add(u // 2)
                diag_sched.setdefault(max(it, 0), []).append(u)

            for b in range(b_per):
                if b + 2 < b_per:
                    emit_load(b + 2)
                emit_convs(b, mid_hook=emit_phase_k if b == 0 else None)
                for u in diag_sched.get(b, ()):
                    build_diag(u)
                emit_xcorr(b)
                if b >= 3:
                    emit_head(b - 3)
            emit_head(b_per - 3)
            emit_head(b_per - 2)
            emit_head(b_per - 1)

    nc.compile()
    return nc


def _get_nc(b_per=B_PER):
    key = b_per
    if key not in _NC_CACHE:
        _NC_CACHE[key] = _build_nc(b_per)
    return _NC_CACHE[key]


def _host_prep(inputs):
    """Fold BN into weights, transpose to lhsT layouts, slice per core."""
    import ml_dtypes

    bf16 = ml_dtypes.bfloat16
    f = np.float32
    kernel = np.ascontiguousarray(inputs["kernel"], dtype=f)
    search = np.ascontiguousarray(inputs["search"], dtype=f)

    def bn_fold(g, b_, m, v):
        scale = g / np.sqrt(v + EPS)
        shift = b_ - m * scale
        return scale.astype(f), shift.astype(f)

    s1, sh1 = bn_fold(inputs["g1"], inputs["b1"], inputs["m1"], inputs["v1"])
    s2, sh2 = bn_fold(inputs["g2"], inputs["b2"], inputs["m2"], inputs["v2"])
    s3, sh3 = bn_fold(inputs["g3"], inputs["b3"], inputs["m3"], inputs["v3"])

    def conv3_lhsT(w, scale):
        # w [co=256, ci=256, 3, 3] * scale[co] -> [cic, ci128, tap*2+coc, co128]
        wf = (w * scale[:, None, None, None]).astype(f)
        wf = wf.reshape(2, 128, 2, 128, 3, 3)  # [coc, co, cic, ci, dy, dx]
        wf = wf.transpose(2, 3, 4, 5, 0, 1)  # [cic, ci, dy, dx, coc, co]
        return np.ascontiguousarray(wf.reshape(2, 128, 18, 128))

    wk = conv3_lhsT(inputs["w_ck"], s1).astype(bf16)
    ws = conv3_lhsT(inputs["w_cs"], s2).astype(bf16)

    w1 = (inputs["w_h1"][:, :, 0, 0] * s3[:, None]).astype(f)  # [co 256, ci 256]
    w1 = w1.reshape(2, 128, 2, 128).transpose(2, 3, 0, 1)  # [cic, ci, coc, co]
    w1 = np.ascontiguousarray(w1)
    w2 = inputs["w_h2"][:, :, 0, 0].astype(f)  # [10, 256]
    w2 = np.ascontiguousarray(w2.reshape(10, 2, 128).transpose(1, 2, 0))  # [cic, ci, 10]

    weights = dict(
        wk=wk,
        ws=ws,
        w1=w1,
        w2=w2,
        eye=np.eye(128, dtype=f),
        b1s=np.ascontiguousarray(sh1.reshape(2, 128).T),
        b2s=np.ascontiguousarray(sh2.reshape(2, 128).T),
        b3s=np.ascontiguousarray(sh3.reshape(2, 128).T),
        bhs=np.ascontiguousarray(inputs["b_h2"].astype(f).reshape(10, 1)),
    )

    in_maps = []
    for c in range(N_CORES):
        sl = slice(c * B_PER, (c + 1) * B_PER)
        win = np.lib.stride_tricks.sliding_window_view(kernel[sl], (5, 5), axis=(2, 3))
        # win[b, c, dy, dx, y, x] = kernel[b, c, y+dy, x+dx]
        kin = win.reshape(B_PER, 2, 128, 9, 25).transpose(1, 2, 3, 0, 4)
        sp = np.zeros((B_PER, 2, 128, 31, 32), dtype=bf16)
        sp[..., :31] = search[sl].reshape(B_PER, 2, 128, 31, 31).astype(bf16)
        m = dict(weights)
        m["search"] = sp
        m["kin"] = np.ascontiguousarray(kin).astype(bf16)
        in_maps.append(m)
    return in_maps


def run(trace=False, **inputs):
    from concourse import bass_utils

    in_maps = _host_prep(inputs)
    nc = _get_nc()
    try:
        res = bass_utils.run_bass_kernel_spmd(
            nc, in_maps, core_ids=list(range(N_CORES)), trace=trace
        )
    except ModuleNotFoundError:
        # NTFF profiling hook unavailable in this container
        res = bass_utils.run_bass_kernel_spmd(
            nc, in_maps, core_ids=list(range(N_CORES)), trace=False
        )
    y = np.concatenate([res.results[c]["y"] for c in range(N_CORES)], axis=0)
    return y.reshape(B, 10, 25, 25), res


def kernel(**inputs):
    y, _ = run(trace=False, **inputs)
    return y


# revision 43
# speedup vs baseline: 1.1339x; 1.0193x over previous
"""Trainium2 Bass kernel for nn_DepthwiseXCorr (SiamRPN-style depthwise-xcorr head).

Pipeline per sample (B=128 sharded 16/core across 8 cores, pure data parallel):
  k = relu(bn1(conv3x3(kernel_in, w_ck)))      [256, 5, 5]
  s = relu(bn2(conv3x3(search_in, w_cs)))      [256, 29, 29]
  feat = depthwise_xcorr(s, k)                 [256, 25, 25]
  h = relu(bn3(conv1x1(feat, w_h1)))           [256, 25, 25]
  out = conv1x1(h, w_h2) + b_h2                [10, 25, 25]

Implementation notes:
  - BN scale folded into conv weights host-side; BN shift + ReLU applied by
    the ACT engine on the PSUM->SBUF eviction (activation = relu(x*1 + bias)).
  - Convs are implicit GEMM on TensorE (fp32r runs 1 cycle/row at free dim
    >= 256, same rate as bf16).
  - Depthwise xcorr (32 (sample,chunk) units/core) is load-balanced across
    engines with an explicit map: DVE and Pool run 25-tap
    scalar_tensor_tensor chains; 7 units (incl. both units of the last
    sample) run on the PE as diagonal-weight matmuls, their diag tiles built
    by the otherwise-idle ACT engine two samples ahead.
  - The head (h1/h2) is software-pipelined 2 samples behind conv/xcorr so
    slow Pool units never stall the PE (PE p-state drops on any stall).
  - DMA ring discipline: search loads on the SP ring only, y stores on the
    ACT ring (issued right after the osb eviction so they never wait),
    weights split across the ACT/DVE rings at startup.
"""

import numpy as np

EPS = 1e-5
N_CORES = 8
B = 128
B_PER = B // N_CORES  # 16
CIN = 256
H = 256
COUT = 10

_NC_CACHE = {}

# unit u = sample*2 + chunk.  GPSIMD supports no scalar_tensor_tensor on real
# HW, so the depthwise xcorr is split PE/DVE only: 14 units run on the PE as
# diagonal-weight matmuls (diag tiles built by Pool broadcast multiplies), 18
# units run as 25-tap scalar_tensor_tensor chains on DVE.
PE_UNITS = (5, 7, 9, 11, 13, 15, 17, 19, 21, 23, 25, 27, 30, 31)


def _build_nc(b_per=B_PER):
    """Build the Bass program for one core processing `b_per` samples."""
    import concourse.bacc as bacc
    import concourse.mybir as mybir
    import concourse.tile as tile

    dt = mybir.dt
    f32 = dt.float32
    f32r = dt.float32r
    AF = mybir.ActivationFunctionType
    ALU = mybir.AluOpType

    nc = bacc.Bacc("TRN2", target_bir_lowering=False, debug=False)

    # ---- DRAM tensors (shapes match SBUF tiles exactly; host pre-transposes) ----
    search_d = nc.dram_tensor("search", [b_per, 2, 128, 31, 32], dt.bfloat16, kind="ExternalInput")
    kin_d = nc.dram_tensor("kin", [2, 128, 9, b_per, 25], dt.bfloat16, kind="ExternalInput")
    wk_d = nc.dram_tensor("wk", [2, 128, 18, 128], dt.bfloat16, kind="ExternalInput")
    ws_d = nc.dram_tensor("ws", [2, 128, 18, 128], dt.bfloat16, kind="ExternalInput")
    w1_d = nc.dram_tensor("w1", [2, 128, 2, 128], f32r, kind="ExternalInput")
    w2_d = nc.dram_tensor("w2", [2, 128, 10], f32r, kind="ExternalInput")
    eye_d = nc.dram_tensor("eye", [128, 128], f32, kind="ExternalInput")
    b1_d = nc.dram_tensor("b1s", [128, 2], f32, kind="ExternalInput")
    b2_d = nc.dram_tensor("b2s", [128, 2], f32, kind="ExternalInput")
    b3_d = nc.dram_tensor("b3s", [128, 2], f32, kind="ExternalInput")
    bh_d = nc.dram_tensor("bhs", [10, 1], f32, kind="ExternalInput")
    y_d = nc.dram_tensor("y", [b_per, 10, 25, 25], f32, kind="ExternalOutput")

    TAPS3 = [(dy, dx) for dy in range(3) for dx in range(3)]
    TAPS5 = [(dy, dx) for dy in range(5) for dx in range(5)]
    # conv_search output row tiling: 29 rows -> two PSUM tiles (N = 435 / 406)
    CS_ROWS = [(0, 15), (15, 14)]
    # h1/h2/PE-xcorr output row tiling: 25 rows -> two PSUM tiles (N = 325 / 300)
    H_ROWS = [(0, 13), (13, 12)]

    with tile.TileContext(nc) as tc:
        with (
            tc.tile_pool(name="wpool", bufs=1) as wpool,
            tc.tile_pool(name="kpool", bufs=1) as kpool,
            tc.tile_pool(name="spool", bufs=6) as spool,
            tc.tile_pool(name="fpool", bufs=10) as fpool,
            tc.tile_pool(name="hpool", bufs=4) as hpool,
            tc.tile_pool(name="sfpool", bufs=8) as sfpool,
            tc.tile_pool(name="opool", bufs=3) as opool,
            tc.tile_pool(name="dpool", bufs=2) as dpool,
            tc.tile_pool(name="ps_cs", bufs=2, space="PSUM") as ps_cs,
            tc.tile_pool(name="ps_h", bufs=4, space="PSUM") as ps_h,
            tc.tile_pool(name="ps_h2", bufs=2, space="PSUM") as ps_h2,
        ):
            sin_tiles = {}

            def emit_load(b):
                sin = []
                for ci in range(2):
                    st = spool.tile([128, 31, 32], dt.bfloat16, tag="sin")
                    nc.sync.dma_start(st[:], search_d[b, ci])
                    sin.append(st)
                sin_tiles[b] = sin

            # ---- startup DMA schedule, ordered by first use:
            #   ACT ring:   kraw0+wk0 (phase K ci=0) + b1, then per-sample y
            #   SP ring:    sin[0], kraw1+wk1 (phase K ci=1), sin[1], head
            #               weights, then the sin prefetch stream
            #   Pool ring:  ws (conv_search) + b2 + eye ----
            kin_sb = []
            wk_sb = []
            ws_sb = []
            w1_sb = []
            w2_sb = []
            # transfers serialize on the DMA engines, so order strictly by
            # first use: conv_search[0] inputs, then phase-K inputs, then the
            # second sample and head weights
            b2_sb = wpool.tile([128, 2], f32, tag="b2")
            nc.scalar.dma_start(b2_sb[:], b2_d[:])
            for c in range(2):
                wst = wpool.tile([128, 18, 128], dt.bfloat16, tag=f"ws{c}")
                nc.gpsimd.dma_start(wst[:], ws_d[c])
                ws_sb.append(wst)
            emit_load(0)
            for c in range(2):
                kt = kpool.tile([128, 9, b_per, 25], dt.bfloat16, tag=f"kin{c}")
                nc.scalar.dma_start(kt[:], kin_d[c])
                kin_sb.append(kt)
                wkt = wpool.tile([128, 18, 128], dt.bfloat16, tag=f"wk{c}")
                nc.gpsimd.dma_start(wkt[:], wk_d[c])
                wk_sb.append(wkt)
            b1_sb = wpool.tile([128, 2], f32, tag="b1")
            nc.scalar.dma_start(b1_sb[:], b1_d[:])
            emit_load(1)
            eye_sb = wpool.tile([128, 128], f32, tag="eye")
            nc.gpsimd.dma_start(eye_sb[:], eye_d[:])
            for c in range(2):
                w1t = wpool.tile([128, 2, 128], f32r, tag=f"w1{c}")
                nc.sync.dma_start(w1t[:], w1_d[c])
                w1_sb.append(w1t)
                w2t = wpool.tile([128, 10], f32r, tag=f"w2{c}")
                nc.sync.dma_start(w2t[:], w2_d[c])
                w2_sb.append(w2t)
            b3_sb = wpool.tile([128, 2], f32, tag="b3")
            nc.sync.dma_start(b3_sb[:], b3_d[:])
            bh_sb = wpool.tile([10, 1], f32, tag="bh")
            nc.sync.dma_start(bh_sb[:], bh_d[:])

            # ---- phase K: conv_kernel for all samples batched (N = b_per*25).
            # Emitted lazily AFTER conv_search[0] so the PE starts on conv
            # while the (larger) kin transfers are still in flight. ----
            kf_sb = []

            def emit_phase_k():
                for cc in range(2):  # output-channel chunk
                    psk = ps_cs.tile([128, b_per, 25], f32, tag="ps")
                    n_acc = len(TAPS3) * 2
                    i = 0
                    for ci in range(2):
                        for (dy, dx) in TAPS3:
                            t2c = (dy * 3 + dx) * 2 + cc
                            nc.tensor.matmul(
                                psk[:],
                                wk_sb[ci][:, t2c, :],
                                kin_sb[ci][:, dy * 3 + dx, :, :],
                                start=(i == 0),
                                stop=(i == n_acc - 1),
                            )
                            i += 1
                    kf = kpool.tile([128, b_per, 25], f32, tag=f"kf{cc}")
                    nc.scalar.activation(kf[:], psk[:], AF.Relu, bias=b1_sb[:, cc : cc + 1])
                    kf_sb.append(kf)

            diag_tiles = {}

            def build_diag(u):
                """diag(kf[:, b, t]) for all 25 taps: dg[c, t, j] = eye[c,j]*kf[c,b,t].

                One Pool broadcast tensor_tensor (~6.5us); Pool is otherwise
                idle so this never contends with evictions or DVE chains."""
                bb, cc = u // 2, u % 2
                dg = dpool.tile([128, 25, 128], f32r, tag="diag")
                mask = eye_sb[:].unsqueeze(1).broadcast_to([128, 25, 128])
                data = kf_sb[cc][:, bb].unsqueeze(2).broadcast_to([128, 25, 128])
                nc.gpsimd.tensor_tensor(dg[:], mask, data, ALU.mult)
                diag_tiles[u] = dg

            sf_tiles = {}
            feat_tiles = {}

            def emit_convs(b, mid_hook=None):
                sin = sin_tiles.pop(b)
                sf = []
                for cc in range(2):
                    if cc == 1 and mid_hook is not None:
                        mid_hook()
                    sft = sfpool.tile([128, 29, 30], f32r, tag="sf")
                    for (r0, nr) in CS_ROWS:
                        pscs = ps_cs.tile([128, 15, 29], f32, tag="ps")
                        n_acc = len(TAPS3) * 2
                        i = 0
                        for ci in range(2):
                            for (dy, dx) in TAPS3:
                                t2c = (dy * 3 + dx) * 2 + cc
                                nc.tensor.matmul(
                                    pscs[:, :nr, :],
                                    ws_sb[ci][:, t2c, :],
                                    sin[ci][:, dy + r0 : dy + r0 + nr, dx : dx + 29],
                                    start=(i == 0),
                                    stop=(i == n_acc - 1),
                                )
                                i += 1
                        nc.scalar.activation(
                            sft[:, r0 : r0 + nr, 0:29],
                            pscs[:, :nr, :],
                            AF.Relu,
                            bias=b2_sb[:, cc : cc + 1],
                        )
                    sf.append(sft)
                sf_tiles[b] = sf

            def emit_xcorr(b):
                sf = sf_tiles.pop(b)
                feat = [None, None]
                for cc in range(2):
                    u = b * 2 + cc
                    ft = fpool.tile([128, 25, 26], f32r, tag="feat")
                    if u in PE_UNITS:
                        dg = diag_tiles.pop(u)
                        for (r0, nr) in H_ROWS:
                            psx = ps_h.tile([128, 13, 26], f32, tag="ph")
                            for ti, (dy, dx) in enumerate(TAPS5):
                                nc.tensor.matmul(
                                    psx[:, :nr, :],
                                    dg[:, ti, :],
                                    sf[cc][:, dy + r0 : dy + r0 + nr, dx : dx + 26],
                                    start=(ti == 0),
                                    stop=(ti == 24),
                                )
                            nc.scalar.activation(
                                ft[:, r0 : r0 + nr, 0:25], psx[:, :nr, 0:25], AF.Copy
                            )
                    else:
                        dv = ft[:, :, 0:25]
                        for ti, (dy, dx) in enumerate(TAPS5):
                            kap = kf_sb[cc][:, b, dy * 5 + dx : dy * 5 + dx + 1]
                            win = sf[cc][:, dy : dy + 25, dx : dx + 25]
                            if ti == 0:
                                nc.vector.tensor_scalar(dv, win, kap, None, ALU.mult)
                            else:
                                nc.vector.scalar_tensor_tensor(
                                    dv, win, kap, dv, ALU.mult, ALU.add
                                )
                    feat[cc] = ft

                feat_tiles[b] = feat

            def emit_head(b):
                feat = feat_tiles.pop(b)
                # h1: 1x1 conv + bn3 + relu -> h1o [2][128, 25, 25]
                h1o = []
                for cc2 in range(2):
                    ht = hpool.tile([128, 25, 26], f32r, tag="h1o")
                    for (r0, nr) in H_ROWS:
                        psh = ps_h.tile([128, 13, 26], f32, tag="ph")
                        for ci in range(2):
                            nc.tensor.matmul(
                                psh[:, :nr, :],
                                w1_sb[ci][:, cc2, :],
                                feat[ci][:, r0 : r0 + nr, :],
                                start=(ci == 0),
                                stop=(ci == 1),
                            )
                        nc.scalar.activation(
                            ht[:, r0 : r0 + nr, 0:25],
                            psh[:, :nr, 0:25],
                            AF.Relu,
                            bias=b3_sb[:, cc2 : cc2 + 1],
                        )
                    h1o.append(ht)

                # h2: 1x1 conv (+bias) -> out [10, 25, 25]
                osb = opool.tile([10, 25, 25], f32, tag="osb")
                for (r0, nr) in H_ROWS:
                    psh2 = ps_h2.tile([10, 13, 26], f32, tag="ph2")
                    for ci in range(2):
                        nc.tensor.matmul(
                            psh2[:, :nr, :],
                            w2_sb[ci][:, :],
                            h1o[ci][:, r0 : r0 + nr, :],
                            start=(ci == 0),
                            stop=(ci == 1),
                        )
                    nc.scalar.activation(
                        osb[:, r0 : r0 + nr, :],
                        psh2[:, :nr, 0:25],
                        AF.Identity,
                        bias=bh_sb[:, :],
                    )
                # y store on the ACT ring: osb was just evicted by ACT, so the
                # DMA dispatch never waits on a semaphore
                nc.scalar.dma_start(y_d[b], osb[:])

            # ---- software-pipelined sample loop (head lags by 2 samples).
            # The head is emitted BEFORE conv_search so its PSUM evictions sit
            # ahead of conv evictions in the in-order ACT queue (no
            # head-of-line blocking -> no PE stall on ps_h banks). ----
            # diag build schedule: 2 iterations ahead of use; when a sample
            # has two PE units, stagger the second to the next iteration so
            # only 2 diag tiles are ever live (dpool bufs=2)
            diag_sched = {}
            seen_samples = set()
            for u in PE_UNITS:
                it = u // 2 - 2
                if u // 2 in seen_samples:
                    it += 1
                seen_samples.add(u // 2)
                diag_sched.setdefault(max(it, 0), []).append(u)

            for b in range(b_per):
                if b + 2 < b_per:
                    emit_load(b + 2)
                emit_convs(b, mid_hook=emit_phase_k if b == 0 else None)
                for u in diag_sched.get(b, ()):
                    build_diag(u)
                emit_xcorr(b)
                if b >= 3:
                    emit_head(b - 3)
            emit_head(b_per - 3)
            emit_head(b_per - 2)
            emit_head(b_per - 1)

    nc.compile()
    return nc


def _get_nc(b_per=B_PER):
    key = b_per
    if key not in _NC_CACHE:
        _NC_CACHE[key] = _build_nc(b_per)
    return _NC_CACHE[key]


def _host_prep(inputs):
    """Fold BN into weights, transpose to lhsT layouts, slice per core."""
    import ml_dtypes

    bf16 = ml_dtypes.bfloat16
    f = np.float32
    kernel = np.ascontiguousarray(inputs["kernel"], dtype=f)
    search = np.ascontiguousarray(inputs["search"], dtype=f)

    def bn_fold(g, b_, m, v):
        scale = g / np.sqrt(v + EPS)
        shift = b_ - m * scale
        return scale.astype(f), shift.astype(f)

    s1, sh1 = bn_fold(inputs["g1"], inputs["b1"], inputs["m1"], inputs["v1"])
    s2, sh2 = bn_fold(inputs["g2"], inputs["b2"], inputs["m2"], inputs["v2"])
    s3, sh3 = bn_fold(inputs["g3"], inputs["b3"], inputs["m3"], inputs["v3"])

    def conv3_lhsT(w, scale):
        # w [co=256, ci=256, 3, 3] * scale[co] -> [cic, ci128, tap*2+coc, co128]
        wf = (w * scale[:, None, None, None]).astype(f)
        wf = wf.reshape(2, 128, 2, 128, 3, 3)  # [coc, co, cic, ci, dy, dx]
        wf = wf.transpose(2, 3, 4, 5, 0, 1)  # [cic, ci, dy, dx, coc, co]
        return np.ascontiguousarray(wf.reshape(2, 128, 18, 128))

    wk = conv3_lhsT(inputs["w_ck"], s1).astype(bf16)
    ws = conv3_lhsT(inputs["w_cs"], s2).astype(bf16)

    w1 = (inputs["w_h1"][:, :, 0, 0] * s3[:, None]).astype(f)  # [co 256, ci 256]
    w1 = w1.reshape(2, 128, 2, 128).transpose(2, 3, 0, 1)  # [cic, ci, coc, co]
    w1 = np.ascontiguousarray(w1)
    w2 = inputs["w_h2"][:, :, 0, 0].astype(f)  # [10, 256]
    w2 = np.ascontiguousarray(w2.reshape(10, 2, 128).transpose(1, 2, 0))  # [cic, ci, 10]

    weights = dict(
        wk=wk,
        ws=ws,
        w1=w1,
        w2=w2,
        eye=np.eye(128, dtype=f),
        b1s=np.ascontiguousarray(sh1.reshape(2, 128).T),
        b2s=np.ascontiguousarray(sh2.reshape(2, 128).T),
        b3s=np.ascontiguousarray(sh3.reshape(2, 128).T),
        bhs=np.ascontiguousarray(inputs["b_h2"].astype(f).reshape(10, 1)),
    )

    in_maps = []
    for c in range(N_CORES):
        sl = slice(c * B_PER, (c + 1) * B_PER)
        win = np.lib.stride_tricks.sliding_window_view(kernel[sl], (5, 5), axis=(2, 3))
        # win[b, c, dy, dx, y, x] = kernel[b, c, y+dy, x+dx]
        kin = win.reshape(B_PER, 2, 128, 9, 25).transpose(1, 2, 3, 0, 4)
        sp = np.zeros((B_PER, 2, 128, 31, 32), dtype=bf16)
        sp[..., :31] = search[sl].reshape(B_PER, 2, 128, 31, 31).astype(bf16)
        m = dict(weights)
        m["search"] = sp
        m["kin"] = np.ascontiguousarray(kin).astype(bf16)
        in_maps.append(m)
    return in_maps


def run(trace=False, **inputs):
    from concourse import bass_utils

    in_maps = _host_prep(inputs)
    nc = _get_nc()
    try:
        res = bass_utils.run_bass_kernel_spmd(
            nc, in_maps, core_ids=list(range(N_CORES)), trace=trace
        )
    except ModuleNotFoundError:
        # NTFF profiling hook unavailable in this container
        res = bass_utils.run_bass_kernel_spmd(
            nc, in_maps, core_ids=list(range(N_CORES)), trace=False
        )
    y = np.concatenate([res.results[c]["y"] for c in range(N_CORES)], axis=0)
    return y.reshape(B, 10, 25, 25), res


def kernel(**inputs):
    y, _ = run(trace=False, **inputs)
    return y


# revision 45
# speedup vs baseline: 1.1404x; 1.0057x over previous
"""Trainium2 Bass kernel for nn_DepthwiseXCorr (SiamRPN-style depthwise-xcorr head).

Pipeline per sample (B=128 sharded 16/core across 8 cores, pure data parallel):
  k = relu(bn1(conv3x3(kernel_in, w_ck)))      [256, 5, 5]
  s = relu(bn2(conv3x3(search_in, w_cs)))      [256, 29, 29]
  feat = depthwise_xcorr(s, k)                 [256, 25, 25]
  h = relu(bn3(conv1x1(feat, w_h1)))           [256, 25, 25]
  out = conv1x1(h, w_h2) + b_h2                [10, 25, 25]

Implementation notes:
  - BN scale folded into conv weights host-side; BN shift + ReLU applied by
    the ACT engine on the PSUM->SBUF eviction (activation = relu(x*1 + bias)).
  - Convs are implicit GEMM on TensorE (fp32r runs 1 cycle/row at free dim
    >= 256, same rate as bf16).
  - Depthwise xcorr (32 (sample,chunk) units/core) is load-balanced across
    engines with an explicit map: DVE and Pool run 25-tap
    scalar_tensor_tensor chains; 7 units (incl. both units of the last
    sample) run on the PE as diagonal-weight matmuls, their diag tiles built
    by the otherwise-idle ACT engine two samples ahead.
  - The head (h1/h2) is software-pipelined 2 samples behind conv/xcorr so
    slow Pool units never stall the PE (PE p-state drops on any stall).
  - DMA ring discipline: search loads on the SP ring only, y stores on the
    ACT ring (issued right after the osb eviction so they never wait),
    weights split across the ACT/DVE rings at startup.
"""

import numpy as np

EPS = 1e-5
N_CORES = 8
B = 128
B_PER = B // N_CORES  # 16
CIN = 256
H = 256
COUT = 10

_NC_CACHE = {}

# unit u = sample*2 + chunk.  GPSIMD supports no scalar_tensor_tensor on real
# HW, so the depthwise xcorr is split PE/DVE only: 14 units run on the PE as
# diagonal-weight matmuls (diag tiles built by Pool broadcast multiplies), 18
# units run as 25-tap scalar_tensor_tensor chains on DVE.
PE_UNITS = (5, 7, 9, 11, 13, 15, 17, 19, 21, 23, 25, 27, 30, 31)


def _build_nc(b_per=B_PER):
    """Build the Bass program for one core processing `b_per` samples."""
    import concourse.bacc as bacc
    import concourse.mybir as mybir
    import concourse.tile as tile

    dt = mybir.dt
    f32 = dt.float32
    f32r = dt.float32r
    AF = mybir.ActivationFunctionType
    ALU = mybir.AluOpType

    nc = bacc.Bacc("TRN2", target_bir_lowering=False, debug=False)

    # ---- DRAM tensors (shapes match SBUF tiles exactly; host pre-transposes) ----
    search_d = nc.dram_tensor("search", [b_per, 2, 128, 31, 32], dt.bfloat16, kind="ExternalInput")
    kin_d = nc.dram_tensor("kin", [2, 128, 9, b_per, 25], dt.bfloat16, kind="ExternalInput")
    wk_d = nc.dram_tensor("wk", [2, 128, 18, 128], dt.bfloat16, kind="ExternalInput")
    ws_d = nc.dram_tensor("ws", [2, 128, 18, 128], dt.bfloat16, kind="ExternalInput")
    w1_d = nc.dram_tensor("w1", [2, 128, 2, 128], f32r, kind="ExternalInput")
    w2_d = nc.dram_tensor("w2", [2, 128, 10], dt.bfloat16, kind="ExternalInput")
    eye_d = nc.dram_tensor("eye", [128, 128], dt.bfloat16, kind="ExternalInput")
    b1_d = nc.dram_tensor("b1s", [128, 2], f32, kind="ExternalInput")
    b2_d = nc.dram_tensor("b2s", [128, 2], f32, kind="ExternalInput")
    b3_d = nc.dram_tensor("b3s", [128, 2], f32, kind="ExternalInput")
    bh_d = nc.dram_tensor("bhs", [10, 1], f32, kind="ExternalInput")
    y_d = nc.dram_tensor("y", [b_per, 10, 25, 25], f32, kind="ExternalOutput")

    TAPS3 = [(dy, dx) for dy in range(3) for dx in range(3)]
    TAPS5 = [(dy, dx) for dy in range(5) for dx in range(5)]
    # conv_search output row tiling: 29 rows -> two PSUM tiles (N = 435 / 406)
    CS_ROWS = [(0, 15), (15, 14)]
    # h1/h2/PE-xcorr output row tiling: 25 rows -> two PSUM tiles (N = 325 / 300)
    H_ROWS = [(0, 13), (13, 12)]

    with tile.TileContext(nc) as tc:
        with (
            tc.tile_pool(name="wpool", bufs=1) as wpool,
            tc.tile_pool(name="kpool", bufs=1) as kpool,
            tc.tile_pool(name="spool", bufs=6) as spool,
            tc.tile_pool(name="fpool", bufs=10) as fpool,
            tc.tile_pool(name="hpool", bufs=4) as hpool,
            tc.tile_pool(name="sfpool", bufs=8) as sfpool,
            tc.tile_pool(name="opool", bufs=3) as opool,
            tc.tile_pool(name="dpool", bufs=2) as dpool,
            tc.tile_pool(name="ps_cs", bufs=2, space="PSUM") as ps_cs,
            tc.tile_pool(name="ps_h", bufs=4, space="PSUM") as ps_h,
            tc.tile_pool(name="ps_h2", bufs=2, space="PSUM") as ps_h2,
        ):
            sin_tiles = {}

            def emit_load(b):
                sin = []
                for ci in range(2):
                    st = spool.tile([128, 31, 32], dt.bfloat16, tag="sin")
                    nc.sync.dma_start(st[:], search_d[b, ci])
                    sin.append(st)
                sin_tiles[b] = sin

            # ---- startup DMA schedule, ordered by first use:
            #   ACT ring:   kraw0+wk0 (phase K ci=0) + b1, then per-sample y
            #   SP ring:    sin[0], kraw1+wk1 (phase K ci=1), sin[1], head
            #               weights, then the sin prefetch stream
            #   Pool ring:  ws (conv_search) + b2 + eye ----
            kin_sb = []
            wk_sb = []
            ws_sb = []
            w1_sb = []
            w2_sb = []
            # transfers serialize on the DMA engines, so order strictly by
            # first use: conv_search[0] inputs, then phase-K inputs, then the
            # second sample and head weights
            b2_sb = wpool.tile([128, 2], f32, tag="b2")
            nc.scalar.dma_start(b2_sb[:], b2_d[:])
            for c in range(2):
                wst = wpool.tile([128, 18, 128], dt.bfloat16, tag=f"ws{c}")
                nc.gpsimd.dma_start(wst[:], ws_d[c])
                ws_sb.append(wst)
            emit_load(0)
            for c in range(2):
                kt = kpool.tile([128, 9, b_per, 25], dt.bfloat16, tag=f"kin{c}")
                nc.scalar.dma_start(kt[:], kin_d[c])
                kin_sb.append(kt)
                wkt = wpool.tile([128, 18, 128], dt.bfloat16, tag=f"wk{c}")
                nc.gpsimd.dma_start(wkt[:], wk_d[c])
                wk_sb.append(wkt)
            b1_sb = wpool.tile([128, 2], f32, tag="b1")
            nc.scalar.dma_start(b1_sb[:], b1_d[:])
            emit_load(1)
            eye_sb = wpool.tile([128, 128], dt.bfloat16, tag="eye")
            nc.gpsimd.dma_start(eye_sb[:], eye_d[:])
            for c in range(2):
                w1t = wpool.tile([128, 2, 128], f32r, tag=f"w1{c}")
                nc.sync.dma_start(w1t[:], w1_d[c])
                w1_sb.append(w1t)
                w2t = wpool.tile([128, 10], dt.bfloat16, tag=f"w2{c}")
                nc.sync.dma_start(w2t[:], w2_d[c])
                w2_sb.append(w2t)
            b3_sb = wpool.tile([128, 2], f32, tag="b3")
            nc.sync.dma_start(b3_sb[:], b3_d[:])
            bh_sb = wpool.tile([10, 1], f32, tag="bh")
            nc.sync.dma_start(bh_sb[:], bh_d[:])

            # ---- phase K: conv_kernel for all samples batched (N = b_per*25).
            # Emitted lazily AFTER conv_search[0] so the PE starts on conv
            # while the (larger) kin transfers are still in flight. ----
            kf_sb = []
            kfb_sb = []

            def emit_phase_k():
                for cc in range(2):  # output-channel chunk
                    psk = ps_cs.tile([128, b_per, 25], f32, tag="ps")
                    n_acc = len(TAPS3) * 2
                    i = 0
                    for ci in range(2):
                        for (dy, dx) in TAPS3:
                            t2c = (dy * 3 + dx) * 2 + cc
                            nc.tensor.matmul(
                                psk[:],
                                wk_sb[ci][:, t2c, :],
                                kin_sb[ci][:, dy * 3 + dx, :, :],
                                start=(i == 0),
                                stop=(i == n_acc - 1),
                            )
                            i += 1
                    kf = kpool.tile([128, b_per, 25], f32, tag=f"kf{cc}")
                    nc.scalar.activation(kf[:], psk[:], AF.Relu, bias=b1_sb[:, cc : cc + 1])
                    kf_sb.append(kf)
                    kfb = kpool.tile([128, b_per, 25], dt.bfloat16, tag=f"kfb{cc}")
                    nc.scalar.activation(kfb[:], psk[:], AF.Relu, bias=b1_sb[:, cc : cc + 1])
                    kfb_sb.append(kfb)

            diag_tiles = {}

            def build_diag(u):
                """diag(kf[:, b, t]) for all 25 taps: dg[c, t, j] = eye[c,j]*kf[c,b,t].

                One Pool broadcast tensor_tensor (~6.5us); Pool is otherwise
                idle so this never contends with evictions or DVE chains."""
                bb, cc = u // 2, u % 2
                dg = dpool.tile([128, 25, 128], dt.bfloat16, tag="diag")
                mask = eye_sb[:].unsqueeze(1).broadcast_to([128, 25, 128])
                data = kfb_sb[cc][:, bb].unsqueeze(2).broadcast_to([128, 25, 128])
                nc.gpsimd.tensor_tensor(dg[:], mask, data, ALU.mult)
                diag_tiles[u] = dg

            sf_tiles = {}
            feat_tiles = {}

            def emit_convs(b, mid_hook=None):
                sin = sin_tiles.pop(b)
                sf = []
                sfb16 = {}
                for cc in range(2):
                    if cc == 1 and mid_hook is not None:
                        mid_hook()
                    on_pe = (b * 2 + cc) in PE_UNITS
                    if on_pe:
                        sbt = sfpool.tile([128, 29, 30], dt.bfloat16, tag="sfb")
                        sfb16[cc] = sbt
                    sft = sfpool.tile([128, 29, 30], f32r, tag="sf")
                    for (r0, nr) in CS_ROWS:
                        pscs = ps_cs.tile([128, 15, 29], f32, tag="ps")
                        n_acc = len(TAPS3) * 2
                        i = 0
                        for ci in range(2):
                            for (dy, dx) in TAPS3:
                                t2c = (dy * 3 + dx) * 2 + cc
                                nc.tensor.matmul(
                                    pscs[:, :nr, :],
                                    ws_sb[ci][:, t2c, :],
                                    sin[ci][:, dy + r0 : dy + r0 + nr, dx : dx + 29],
                                    start=(i == 0),
                                    stop=(i == n_acc - 1),
                                )
                                i += 1
                        nc.scalar.activation(
                            sft[:, r0 : r0 + nr, 0:29],
                            pscs[:, :nr, :],
                            AF.Relu,
                            bias=b2_sb[:, cc : cc + 1],
                        )
                        if on_pe:
                            nc.scalar.activation(
                                sfb16[cc][:, r0 : r0 + nr, 0:29],
                                pscs[:, :nr, :],
                                AF.Relu,
                                bias=b2_sb[:, cc : cc + 1],
                            )
                    sf.append(sft)
                sf_tiles[b] = (sf, sfb16)

            def emit_xcorr(b):
                sf, sfb16 = sf_tiles.pop(b)
                feat = [None, None]
                for cc in range(2):
                    u = b * 2 + cc
                    ft = fpool.tile([128, 25, 26], f32r, tag="feat")
                    if u in PE_UNITS:
                        dg = diag_tiles.pop(u)
                        for (r0, nr) in H_ROWS:
                            psx = ps_h.tile([128, 13, 25], f32, tag="ph")
                            for ti, (dy, dx) in enumerate(TAPS5):
                                nc.tensor.matmul(
                                    psx[:, :nr, :],
                                    dg[:, ti, :],
                                    sfb16[cc][:, dy + r0 : dy + r0 + nr, dx : dx + 25],
                                    start=(ti == 0),
                                    stop=(ti == 24),
                                )
                            nc.scalar.activation(
                                ft[:, r0 : r0 + nr, 0:25], psx[:, :nr, :], AF.Copy
                            )
                    else:
                        dv = ft[:, :, 0:25]
                        for ti, (dy, dx) in enumerate(TAPS5):
                            kap = kf_sb[cc][:, b, dy * 5 + dx : dy * 5 + dx + 1]
                            win = sf[cc][:, dy : dy + 25, dx : dx + 25]
                            if ti == 0:
                                nc.vector.tensor_scalar(dv, win, kap, None, ALU.mult)
                            else:
                                nc.vector.scalar_tensor_tensor(
                                    dv, win, kap, dv, ALU.mult, ALU.add
                                )
                    feat[cc] = ft

                feat_tiles[b] = feat

            def emit_head(b):
                feat = feat_tiles.pop(b)
                # h1: 1x1 conv + bn3 + relu -> h1o [2][128, 25, 25]
                h1o = []
                for cc2 in range(2):
                    ht = hpool.tile([128, 25, 25], dt.bfloat16, tag="h1o")
                    for (r0, nr) in H_ROWS:
                        psh = ps_h.tile([128, 13, 26], f32, tag="ph")
                        for ci in range(2):
                            nc.tensor.matmul(
                                psh[:, :nr, :],
                                w1_sb[ci][:, cc2, :],
                                feat[ci][:, r0 : r0 + nr, :],
                                start=(ci == 0),
                                stop=(ci == 1),
                            )
                        nc.scalar.activation(
                            ht[:, r0 : r0 + nr, :],
                            psh[:, :nr, 0:25],
                            AF.Relu,
                            bias=b3_sb[:, cc2 : cc2 + 1],
                        )
                    h1o.append(ht)

                # h2: 1x1 conv (+bias) -> out [10, 25, 25]
                osb = opool.tile([10, 25, 25], f32, tag="osb")
                for (r0, nr) in H_ROWS:
                    psh2 = ps_h2.tile([10, 13, 25], f32, tag="ph2")
                    for ci in range(2):
                        nc.tensor.matmul(
                            psh2[:, :nr, :],
                            w2_sb[ci][:, :],
                            h1o[ci][:, r0 : r0 + nr, :],
                            start=(ci == 0),
                            stop=(ci == 1),
                        )
                    nc.scalar.activation(
                        osb[:, r0 : r0 + nr, :],
                        psh2[:, :nr, :],
                        AF.Identity,
                        bias=bh_sb[:, :],
                    )
                # y store on the ACT ring: osb was just evicted by ACT, so the
                # DMA dispatch never waits on a semaphore
                nc.scalar.dma_start(y_d[b], osb[:])

            # ---- software-pipelined sample loop (head lags by 2 samples).
            # The head is emitted BEFORE conv_search so its PSUM evictions sit
            # ahead of conv evictions in the in-order ACT queue (no
            # head-of-line blocking -> no PE stall on ps_h banks). ----
            # diag build schedule: 2 iterations ahead of use; when a sample
            # has two PE units, stagger the second to the next iteration so
            # only 2 diag tiles are ever live (dpool bufs=2)
            diag_sched = {}
            seen_samples = set()
            for u in PE_UNITS:
                it = u // 2 - 2
                if u // 2 in seen_samples:
                    it += 1
                seen_samples.add(u // 2)
                diag_sched.setdefault(max(it, 0), []).append(u)

            for b in range(b_per):
                if b + 2 < b_per:
                    emit_load(b + 2)
                emit_convs(b, mid_hook=emit_phase_k if b == 0 else None)
                for u in diag_sched.get(b, ()):
                    build_diag(u)
                if b == b_per - 1:
                    # tail: drain two heads before the last xcorr so their
                    # PSUM ping-pong overlaps the xcorr matmuls
                    emit_head(b - 3)
                    emit_head(b - 2)
                    emit_xcorr(b)
                    emit_head(b - 1)
                    emit_head(b)
                else:
                    emit_xcorr(b)
                    if b >= 3:
                        emit_head(b - 3)

    nc.compile()
    return nc


def _get_nc(b_per=B_PER):
    key = b_per
    if key not in _NC_CACHE:
        _NC_CACHE[key] = _build_nc(b_per)
    return _NC_CACHE[key]


def _host_prep(inputs):
    """Fold BN into weights, transpose to lhsT layouts, slice per core."""
    import ml_dtypes

    bf16 = ml_dtypes.bfloat16
    f = np.float32
    kernel = np.ascontiguousarray(inputs["kernel"], dtype=f)
    search = np.ascontiguousarray(inputs["search"], dtype=f)

    def bn_fold(g, b_, m, v):
        scale = g / np.sqrt(v + EPS)
        shift = b_ - m * scale
        return scale.astype(f), shift.astype(f)

    s1, sh1 = bn_fold(inputs["g1"], inputs["b1"], inputs["m1"], inputs["v1"])
    s2, sh2 = bn_fold(inputs["g2"], inputs["b2"], inputs["m2"], inputs["v2"])
    s3, sh3 = bn_fold(inputs["g3"], inputs["b3"], inputs["m3"], inputs["v3"])

    def conv3_lhsT(w, scale):
        # w [co=256, ci=256, 3, 3] * scale[co] -> [cic, ci128, tap*2+coc, co128]
        wf = (w * scale[:, None, None, None]).astype(f)
        wf = wf.reshape(2, 128, 2, 128, 3, 3)  # [coc, co, cic, ci, dy, dx]
        wf = wf.transpose(2, 3, 4, 5, 0, 1)  # [cic, ci, dy, dx, coc, co]
        return np.ascontiguousarray(wf.reshape(2, 128, 18, 128))

    wk = conv3_lhsT(inputs["w_ck"], s1).astype(bf16)
    ws = conv3_lhsT(inputs["w_cs"], s2).astype(bf16)

    w1 = (inputs["w_h1"][:, :, 0, 0] * s3[:, None]).astype(f)  # [co 256, ci 256]
    w1 = w1.reshape(2, 128, 2, 128).transpose(2, 3, 0, 1)  # [cic, ci, coc, co]
    w1 = np.ascontiguousarray(w1)
    w2 = inputs["w_h2"][:, :, 0, 0].astype(f)  # [10, 256]
    w2 = np.ascontiguousarray(w2.reshape(10, 2, 128).transpose(1, 2, 0)).astype(bf16)  # [cic, ci, 10]

    weights = dict(
        wk=wk,
        ws=ws,
        w1=w1,
        w2=w2,
        eye=np.eye(128, dtype=bf16),
        b1s=np.ascontiguousarray(sh1.reshape(2, 128).T),
        b2s=np.ascontiguousarray(sh2.reshape(2, 128).T),
        b3s=np.ascontiguousarray(sh3.reshape(2, 128).T),
        bhs=np.ascontiguousarray(inputs["b_h2"].astype(f).reshape(10, 1)),
    )

    in_maps = []
    for c in range(N_CORES):
        sl = slice(c * B_PER, (c + 1) * B_PER)
        win = np.lib.stride_tricks.sliding_window_view(kernel[sl], (5, 5), axis=(2, 3))
        # win[b, c, dy, dx, y, x] = kernel[b, c, y+dy, x+dx]
        kin = win.reshape(B_PER, 2, 128, 9, 25).transpose(1, 2, 3, 0, 4)
        sp = np.zeros((B_PER, 2, 128, 31, 32), dtype=bf16)
        sp[..., :31] = search[sl].reshape(B_PER, 2, 128, 31, 31).astype(bf16)
        m = dict(weights)
        m["search"] = sp
        m["kin"] = np.ascontiguousarray(kin).astype(bf16)
        in_maps.append(m)
    return in_maps


def run(trace=False, **inputs):
    from concourse import bass_utils

    in_maps = _host_prep(inputs)
    nc = _get_nc()
    try:
        res = bass_utils.run_bass_kernel_spmd(
            nc, in_maps, core_ids=list(range(N_CORES)), trace=trace
        )
    except ModuleNotFoundError:
        # NTFF profiling hook unavailable in this container
        res = bass_utils.run_bass_kernel_spmd(
            nc, in_maps, core_ids=list(range(N_CORES)), trace=False
        )
    y = np.concatenate([res.results[c]["y"] for c in range(N_CORES)], axis=0)
    return y.reshape(B, 10, 25, 25), res


def kernel(**inputs):
    y, _ = run(trace=False, **inputs)
    return y
